# revision 16
# baseline (speedup 1.0000x reference)
"""Sparse BertSelfAttention on 8 trn2 NeuronCores.

Sharding: core c -> batch b = c//4, head-group g = c%4 (heads 4g..4g+3).
Each core computes its batch's QT/KT/V projections for its 4 heads and the
sparse attention (local 128-band + global summary columns), producing the
output column block [2048, 256] for its (batch, head-group) in TRANSPOSED,
UNNORMALIZED form; the host divides by the softmax denominators and
transposes while assembling the full output.

Sparse structure (STRIDE=128, EXPR=8, L=2048, bidirectional):
  Query q's local key block is b*(q) = (q-1)//128 (q>=1); b*(0) = 0.
  So the SHIFTED query window w = [128w+1, 128w+129) attends key block w
  fully dense, plus the global summary columns:
    A: cols with (c mod 128) in 120..127  (128 cols)
    B: cols 128, 256, ..., 1920           (15 cols)
  Double-count handling: A-dup rows inside the local tile get a NEG bias
  folded into the Exp activation; the B-dup (local row 0, windows >= 1) is
  excluded by zeroing V row 0 of those blocks (adds 0 to numerator AND
  denominator). Query row 0 (keys = block 0 + globals) is computed on the
  HOST (tiny gemm over the ~271 allowed keys).

Layout: scores are computed transposed (S^T[k, q], keys on partitions) so
softmax denominators come from a ones-column appended to V. The context is
ALSO computed transposed -- ctx^T[d, q] = V-stationary matmuls with the
probability tiles as the moving operand -- which amortizes the global-A/B
contributions over 512 queries per matmul and makes the output DMA a
65-descriptor per-partition-contiguous transfer.
exp() skips max-subtraction: allowed scores are O(5), masked underflow to 0.

HW scheduling rules this file encodes (measured on the device, A/B in the
same benchmark run; the axon-tunnelled part drifts ~20% across runs so only
same-run comparisons were trusted):
 * Strided input DMAs are descriptor-generation-bound (~19ns/descriptor):
   wq/wv are host-packed to the exact SBUF layout so every input DMA is one
   descriptor per partition (input stream alone: ~45us -> ~6us/iter).
 * The transposed output turns 17 x 128-descriptor window stores into 4
   65-descriptor per-qc stores, and removes the reciprocal/normalize DVE
   work from the device entirely.
 * The gathered global K^T columns are plain columns of kt: gathered with
   4 DVE copies instead of 16 PE matmuls + 2 activations.
 * V projections run c-inner (8-chunk accumulation per block, 2-bank
   rotation); a 4-bank c-outer wave variant measured ~20% SLOWER end to
   end (v_mode="wave4" kept for reference). PV interleaves head pairs
   across two PSUM banks ("pair"), which beat per-head chains ("seq").
 * No per-iteration warmup matmuls: they occupied a PSUM bank used by the
   PV tail and serialized the iteration boundary.

Projections stream ht chunk-outer (c-outer) against 8 concurrent PSUM
accumulators so the PE starts as soon as the first 128-row chunk of ht
lands (each chunk's slice of Wk is packed in front of it so one DMA +
one semaphore covers both).
"""

import numpy as np
import ml_dtypes

import concourse.bass as bass
from concourse import bacc
import concourse.mybir as mybir
import concourse.tile as tile
from concourse.bass_utils import run_bass_kernel_spmd

BF16 = mybir.dt.bfloat16
F32 = mybir.dt.float32
AF = mybir.ActivationFunctionType

L = 2048
HID = 1024
NB = L // 128  # 16 key blocks / query windows
NEG = -10000.0

_prog_cache = {}


def _rep_attnB(v):
    out = np.zeros((128, 1), np.float32)
    for h in range(4):
        out[32 * h : 32 * h + 15, 0] = v
    return out


def _glob_cols():
    # A: (16 blocks) x (8 cols 120..127); B: 128,256,...,1920
    a = (np.arange(16)[:, None] * 128 + 120 + np.arange(8)[None, :]).reshape(-1)
    b = np.arange(1, 16) * 128
    return a, b


def _emit_body(nc, consts, pp, psum, parts, am_zero, ht_d, wq_d, wv_d, sm_d,
               out_d, warmup=False, v_mode="inner2", pv_mode="pair"):
    # ---- input DMAs (all single-descriptor-per-partition) ----
    hwl = [
        consts.tile([128, 2304], BF16, tag=f"hw{c}", name=f"hw{c}")
        for c in range(8)
    ]
    htl = [hw[:, 256:2304] for hw in hwl]
    wq = consts.tile([128, 8, 256], BF16)
    wv = consts.tile([128, 8, 260], BF16)
    smalls = consts.tile([128, 6 + NB], F32)
    bqk = smalls[:, 0:4]
    aA = smalls[:, 4:5]
    aB = smalls[:, 5:6]
    bS = smalls[:, 6 : 6 + NB]

    for c in range(8):
        r = slice(128 * c, 128 * c + 128)
        if c == 0:
            nc.sync.dma_start(out=hwl[c][:, 0:1280], in_=ht_d[r, 0:1280])
            nc.sync.dma_start(out=hwl[c][:, 1280:2304], in_=ht_d[r, 1280:2304])
        else:
            nc.sync.dma_start(out=hwl[c], in_=ht_d[r, :])
    nc.sync.dma_start(out=wq.rearrange("p c n -> p (c n)"), in_=wq_d[:, :])
    nc.scalar.dma_start(out=wv.rearrange("p c n -> p (c n)"), in_=wv_d[:, :])
    nc.scalar.dma_start(out=smalls, in_=sm_d[:, :])

    # compact copies of ht's global summary columns (feed the vgA/vgB
    # projections; matmul operands need a single free dim).
    # cols 0:128 = A, 128:143 = B, 143:160 zero-pad.
    ghtAB = consts.tile([128, 8, 160], BF16)
    nc.vector.memset(ghtAB.rearrange("p a b -> p (a b)"), 0.0)
    for c in range(8):
        src = htl[c].rearrange("p (a b) -> p a b", b=128)
        nc.vector.tensor_copy(
            ghtAB[:, c, 0:128].rearrange("p (a b) -> p a b", b=8),
            src[:, :, 120:128],
        )
        nc.vector.tensor_copy(ghtAB[:, c, 128:143], src[:, 1:16, 0])

    def P(i):
        # 8 statically-tagged PSUM bank slots, reused across phases
        # (per-tag WAR deps give fine-grained overlap).
        return psum.tile([128, 512], F32, tag=f"P{i % 8}", name=f"P{i % 8}")

    # PE warmup (p-state ramp) while the first input chunks land.
    if warmup:
        dummy = consts.tile([128, 512], BF16)
        nc.vector.memset(dummy, 0.0)
        warm = P(7)
        for _ in range(5):
            nc.tensor.matmul(warm, lhsT=dummy[:, 0:128], rhs=dummy,
                             start=True, stop=True)

    # ---- K then Q projections, chunk-outer over ht ----
    qtl = [consts.tile([128, L], BF16, tag=f"qt{t}", name=f"qt{t}")
           for t in range(2)]
    ktl = [consts.tile([128, L], BF16, tag=f"kt{t}", name=f"kt{t}")
           for t in range(2)]
    for dstl, wsrc, bcol in (
        ((ktl, "hw", 2), (qtl, "wq", 0)) if "proj" in parts else ()
    ):
        tiles = {}
        for c in range(8):
            for t in range(2):
                for n in range(4):
                    if c == 0:
                        tiles[t, n] = P(4 * t + n)
                    lhsT = (
                        hwl[c][:, 128 * t : 128 * t + 128]
                        if wsrc == "hw"
                        else wq[:, c, 128 * t : 128 * t + 128]
                    )
                    nc.tensor.matmul(
                        tiles[t, n],
                        lhsT=lhsT,
                        rhs=htl[c][:, 512 * n : 512 * n + 512],
                        start=(c == 0),
                        stop=(c == 7),
                    )
        for t in range(2):
            for n in range(4):
                nc.scalar.activation(
                    dstl[t][:, 512 * n : 512 * n + 512],
                    tiles[t, n],
                    AF.Identity,
                    bias=bqk[:, bcol + t : bcol + t + 1],
                )

    # ---- gathered global K^T (plain columns of kt) and global V ----
    ktgAB = consts.tile([128, 2, 160], BF16)
    vgA = consts.tile([128, 260], BF16)
    vgB = consts.tile([128, 260], BF16)
    if "ktg" in parts:
        for t in range(2):
            src = ktl[t].rearrange("p (a b) -> p a b", b=128)
            nc.vector.tensor_copy(
                ktgAB[:, t, 0:128].rearrange("p (a b) -> p a b", b=8),
                src[:, :, 120:128],
            )
            nc.vector.tensor_copy(ktgAB[:, t, 128:143], src[:, 1:16, 0])
            nc.vector.memset(ktgAB[:, t, 143:160], 0.0)

        # vgA/vgB pair-interleaved across two banks (same-bank
        # back-to-back accumulation stalls the PE)
        psa, psb = P(4), P(5)
        for c in range(8):
            nc.tensor.matmul(
                psa[:, 0:260], lhsT=ghtAB[:, c, 0:128], rhs=wv[:, c, :],
                start=(c == 0), stop=(c == 7),
            )
            nc.tensor.matmul(
                psb[0:32, 0:260], lhsT=ghtAB[:, c, 128:160], rhs=wv[:, c, :],
                start=(c == 0), stop=(c == 7),
            )
        nc.vector.tensor_copy(vgA, psa[:, 0:260])
        nc.vector.memset(
            vgA.rearrange("p (h d) -> p h d", d=65)[:, :, 64:65], 1.0
        )
        nc.vector.tensor_copy(vgB[0:15, :], psb[0:15, 0:260])
        nc.vector.memset(
            vgB[0:15, :].rearrange("p (h d) -> p h d", d=65)[:, :, 64:65], 1.0
        )
        for h_ in range(1, 4):
            nc.sync.dma_start(
                out=vgB[32 * h_ : 32 * h_ + 15, :], in_=vgB[0:15, :]
            )

    # ---- attention: V waves, scores/exp, transposed PV ----
    vl = [consts.tile([128, 260], BF16, tag=f"v{blk}", name=f"v{blk}")
          for blk in range(NB)]
    outT = consts.tile([128, 4, 4, 512], F32, tag="outT", name="outT")
    ns = [0]
    scores_of = {}

    def emit_V(w, nbank=4):
        # nbank blocks per wave, c-outer across nbank banks
        blks = range(nbank * w, nbank * w + nbank)
        tiles = {}
        for c in range(8):
            for i, blk in enumerate(blks):
                if c == 0:
                    tiles[i] = P(i)
                nc.tensor.matmul(
                    tiles[i][:, 0:260],
                    lhsT=htl[c][:, 128 * blk : 128 * blk + 128],
                    rhs=wv[:, c, :],
                    start=(c == 0), stop=(c == 7),
                )
        for i, blk in enumerate(blks):
            nc.vector.tensor_copy(vl[blk], tiles[i][:, 0:260])
            nc.vector.memset(
                vl[blk].rearrange("p (h d) -> p h d", d=65)[:, :, 64:65], 1.0
            )
            # B-dup exclusion (key col 128*blk, windows >= 1) by zeroing
            # V row 0: adds 0 to numerator AND denominator, so the S-exp
            # bias is window-invariant
            if blk >= 1:
                nc.gpsimd.memset(vl[blk][0:1, :], 0.0)

    def emit_scores(qc):
        lo = 512 * qc + 1
        W = 512 if qc < 3 else 511
        slots = (4, 5, 0, 1)
        # B-scores for all 4 heads (packed on partition strips; unwritten
        # strips never read downstream)
        pgB = P(6)
        for h in range(4):
            t, hh = h // 2, h % 2
            p0 = 64 * hh
            nc.tensor.matmul(
                pgB[32 * h : 32 * h + 32, 0:W],
                lhsT=ktgAB[p0 : p0 + 64, t, 128:160],
                rhs=qtl[t][p0 : p0 + 64, lo : lo + W],
                start=True, stop=True,
                tile_position=(p0, 32 * h),
            )
        pB = pp.tile([128, 512], BF16, tag="pB")
        nc.scalar.activation(pB[:, 0:W], pgB[:, 0:W], AF.Exp, bias=aB)

        pAs, pSs = [], []
        for h in range(4):
            t, hh = h // 2, h % 2
            p0 = 64 * hh

            # global-A scores + exp
            pgA = P(slots[ns[0] % len(slots)])
            ns[0] += 1
            nc.tensor.matmul(
                pgA[:, 0:W],
                lhsT=ktgAB[p0 : p0 + 64, t, 0:128],
                rhs=qtl[t][p0 : p0 + 64, lo : lo + W],
                start=True, stop=True,
            )
            pA = pp.tile([128, 512], BF16, tag="pA")
            nc.scalar.activation(pA[:, 0:W], pgA[:, 0:W], AF.Exp, bias=aA)

            # local window scores: window w keys = block w
            pss = P(slots[ns[0] % len(slots)])
            ns[0] += 1
            for j in range(4):
                w_ = 4 * qc + j
                wW = 128 if w_ < 15 else 127
                nc.tensor.matmul(
                    pss[:, 128 * j : 128 * j + wW],
                    lhsT=ktl[t][p0 : p0 + 64, 128 * w_ : 128 * w_ + 128],
                    rhs=qtl[t][p0 : p0 + 64,
                               128 * w_ + 1 : 128 * w_ + 1 + wW],
                    start=True, stop=True,
                )
            pS = pp.tile([128, 512], BF16, tag="pS")
            if am_zero:
                # row-0 exclusion lives in the zeroed V rows, so one bias
                # column serves every window
                nc.scalar.activation(pS[:, 0:W], pss[:, 0:W], AF.Exp,
                                     bias=bS[:, 0:1])
            else:
                for j in range(4):
                    w_ = 4 * qc + j
                    wW = 128 if w_ < 15 else 127
                    nc.scalar.activation(
                        pS[:, 128 * j : 128 * j + wW],
                        pss[:, 128 * j : 128 * j + wW],
                        AF.Exp, bias=bS[:, w_ : w_ + 1],
                    )
            pAs.append(pA)
            pSs.append(pS)

        scores_of[qc] = (pB, pAs, pSs)

    def emit_pv(qc):
        # ctx^T[d, q]: V stationary, probs moving; head pairs interleave
        # across two banks so no same-bank back-to-back accumulation.
        pB, pAs, pSs = scores_of.pop(qc)
        W = 512 if qc < 3 else 511
        for hp in (0, 1):
            cxs = []
            for i, h in enumerate((2 * hp, 2 * hp + 1)):
                cxt = P(7 if i == 0 else 3)
                cxs.append(cxt[0:65, 0:512])
            hs_order = (
                [(s_, i_) for s_ in range(6) for i_ in (0, 1)]
                if pv_mode == "pair" else
                [(s_, i_) for i_ in (0, 1) for s_ in range(6)]
            )
            for stage, i in hs_order:
                    h = 2 * hp + i
                    cx = cxs[i]
                    if stage == 0:
                        nc.tensor.matmul(
                            cx[:, 0:W],
                            lhsT=vgA[:, 65 * h : 65 * h + 65],
                            rhs=pAs[h][:, 0:W],
                            start=True, stop=False,
                            skip_group_check=True,
                        )
                    elif stage == 1:
                        nc.tensor.matmul(
                            cx[:, 0:W],
                            lhsT=vgB[32 * h : 32 * h + 15,
                                     65 * h : 65 * h + 65],
                            rhs=pB[32 * h : 32 * h + 15, 0:W],
                            start=False, stop=False,
                            tile_position=(32 * h, 0),
                            skip_group_check=True,
                        )
                    else:
                        j = stage - 2
                        w_ = 4 * qc + j
                        wW = 128 if w_ < 15 else 127
                        nc.tensor.matmul(
                            cx[:, 128 * j : 128 * j + wW],
                            lhsT=vl[w_][:, 65 * h : 65 * h + 65],
                            rhs=pSs[h][:, 128 * j : 128 * j + wW],
                            start=False, stop=(j == 3),
                            skip_group_check=True,
                        )
            for i, h in enumerate((2 * hp, 2 * hp + 1)):
                nc.vector.tensor_copy(outT[0:65, qc, h, 0:W], cxs[i][:, 0:W])
        q_ = nc.sync if qc % 2 == 0 else nc.scalar
        q_.dma_start(out=out_d[qc, :, :, :], in_=outT[0:65, qc, :, :])

    # V waves all run right after the projections: the last htl readers
    # then finish mid-iteration, so the next iteration's input DMAs overlap
    # the whole scores/PV tail (input stream is free in steady state).
    do_v = "v" in parts
    do_s = "scores" in parts
    do_p = do_s and "pv" in parts
    if do_v:
        if v_mode == "wave8":
            for w in range(2):
                emit_V(w, nbank=8)
        elif v_mode == "inner2":
            for blk in range(16):
                ps_ = P(blk % 2)
                for c in range(8):
                    nc.tensor.matmul(
                        ps_[:, 0:260],
                        lhsT=htl[c][:, 128 * blk : 128 * blk + 128],
                        rhs=wv[:, c, :],
                        start=(c == 0), stop=(c == 7),
                    )
                nc.vector.tensor_copy(vl[blk], ps_[:, 0:260])
                nc.vector.memset(
                    vl[blk].rearrange("p (h d) -> p h d", d=65)[:, :, 64:65],
                    1.0,
                )
                if blk >= 1:
                    nc.gpsimd.memset(vl[blk][0:1, :], 0.0)
        else:
            for w in range(4):
                emit_V(w)
    if do_s: emit_scores(0)
    if do_s: emit_scores(1)
    if do_p: emit_pv(0)
    if do_s: emit_scores(2)
    if do_p: emit_pv(1)
    if do_s: emit_scores(3)
    if do_p: emit_pv(2)
    if do_p: emit_pv(3)


def build_program(loop_n=None, am_zero=True, reps=1, warmup=False,
                  v_mode="inner2", pv_mode="pair",
                  parts=("proj", "ktg", "v", "scores", "pv")):
    nc = bacc.Bacc(None)
    # ht chunks with the matching wk chunk packed in front: [wk 256 | ht 2048]
    ht_d = nc.dram_tensor("htwk", [HID, 2304], BF16, kind="ExternalInput")
    wq_d = nc.dram_tensor("wqp", [128, 8 * 256], BF16, kind="ExternalInput")
    wv_d = nc.dram_tensor("wvp", [128, 8 * 260], BF16, kind="ExternalInput")
    # smalls: cols 0:4 bqk | 4:5 attnA | 5:6 attnB | 6:22 biasS
    sm_d = nc.dram_tensor("smalls", [128, 6 + NB], F32, kind="ExternalInput")
    # out[qc][p][h][c] = ctx^T (d=p, head h, query 512*qc+1+c); p=64 rows are
    # the softmax denominators. Host divides + transposes.
    out_d = nc.dram_tensor("out", [4, 65, 4, 512], F32, kind="ExternalOutput")

    with tile.TileContext(nc) as tc:
        with (
            tc.tile_pool(name="consts", bufs=1) as consts,
            tc.tile_pool(name="pp", bufs=8) as pp,
            tc.tile_pool(name="ps", bufs=1, space="PSUM") as psum,
        ):
            import contextlib
            _lp = tc.For_i(0, loop_n, 1) if loop_n else contextlib.nullcontext()
            with _lp:
                for _rep in range(reps):
                    _emit_body(nc, consts, pp, psum, parts, am_zero,
                               ht_d, wq_d, wv_d, sm_d, out_d, warmup=warmup,
                               v_mode=v_mode, pv_mode=pv_mode)
    nc.finalize()
    return nc


def _prepare_inputs(hidden_states, attention_mask, Wq, bq, Wk, bk, Wv, bv,
                    sparse_mask):
    bf = ml_dtypes.bfloat16
    hs = np.asarray(hidden_states, np.float32)
    am = np.asarray(attention_mask, np.float32).reshape(2, L)
    Wq = np.asarray(Wq, np.float32)
    Wk = np.asarray(Wk, np.float32)
    Wv = np.asarray(Wv, np.float32)
    bq = np.asarray(bq, np.float32)
    bk = np.asarray(bk, np.float32)
    gA, gB = _glob_cols()

    in_maps = []
    per_batch = {}
    for b in range(2):
        ht = hs[b].T.astype(np.float32)  # [1024, 2048]
        # per-window local bias: am over the window's key block, globals
        # excluded (A rows always; k=0 is in B for windows >= 1)
        bS = np.empty((128, NB), np.float32)
        for w in range(NB):
            col = am[b][128 * w : 128 * w + 128].copy()
            col[120:128] = NEG
            if w >= 1:
                col[0] = NEG
            bS[:, w] = col
        per_batch[b] = (
            ht,
            bS,
            am[b][gA].reshape(128, 1).copy(),
            _rep_attnB(am[b][gB]),
        )

    for core in range(8):
        b, g = core // 4, core % 4
        ht, bS, aAv, aBv = per_batch[b]
        cols = slice(256 * g, 256 * g + 256)
        # wq/wv packed chunk-major to the exact SBUF tile layout so the
        # input DMA is one descriptor per partition.
        wqp = np.ascontiguousarray(
            (Wq[:, cols] * 0.125).reshape(8, 128, 256).transpose(1, 0, 2)
            .reshape(128, 8 * 256)
        ).astype(bf)
        htwk = np.empty((HID, 2304), np.float32)
        htwk[:, 0:256] = Wk[:, cols]
        htwk[:, 256:2304] = ht
        wv_ = np.zeros((HID, 260), np.float32)
        for j in range(4):
            wv_[:, 65 * j : 65 * j + 64] = (
                Wv[:, cols.start + 64 * j : cols.start + 64 * j + 64]
            )
        wvp = np.ascontiguousarray(
            wv_.reshape(8, 128, 260).transpose(1, 0, 2).reshape(128, 8 * 260)
        ).astype(bf)
        bqk_ = np.stack(
            [
                bq[cols][:128] * 0.125,
                bq[cols][128:] * 0.125,
                bk[cols][:128],
                bk[cols][128:],
            ],
            axis=1,
        ).astype(np.float32)
        sm = np.empty((128, 6 + NB), np.float32)
        sm[:, 0:4] = bqk_
        sm[:, 4:5] = aAv
        sm[:, 5:6] = aBv
        sm[:, 6 : 6 + NB] = bS
        in_maps.append(
            dict(
                htwk=htwk.astype(bf),
                wqp=wqp,
                wvp=wvp,
                smalls=np.ascontiguousarray(sm),
            )
        )
    # NOTE: bv is folded nowhere: it is zeros by construction in this problem.
    assert np.all(np.asarray(bv) == 0.0), "kernel assumes zero V bias"
    return in_maps


def _host_row0(hidden_states, attention_mask, Wq, bq, Wk, bk, Wv, bv,
               sparse_mask):
    """Output row 0 (query position 0) for both batches, exact reference
    math restricted to the ~271 allowed keys of mask row 0."""
    hs = np.asarray(hidden_states, np.float32)
    am = np.asarray(attention_mask, np.float32).reshape(2, L)
    mask_row = np.asarray(sparse_mask, np.float32)[0]
    allowed = np.where(mask_row == 0.0)[0]
    Wq = np.asarray(Wq, np.float32)
    Wk = np.asarray(Wk, np.float32)
    Wv = np.asarray(Wv, np.float32)
    out0 = np.empty((2, HID), np.float32)
    for b in range(2):
        q0 = hs[b, 0] @ Wq + np.asarray(bq, np.float32)
        sub = hs[b][allowed]
        Ks = sub @ Wk + np.asarray(bk, np.float32)
        Vs = sub @ Wv + np.asarray(bv, np.float32)
        for h in range(16):
            sl = slice(64 * h, 64 * h + 64)
            s = np.clip(Ks[:, sl] @ q0[sl] / 8.0, -1e4, 1e4) + am[b][allowed]
            e = np.exp(s - s.max())
            out0[b, sl] = (e / e.sum()) @ Vs[:, sl]
    return out0


def kernel(hidden_states, attention_mask, Wq, bq, Wk, bk, Wv, bv, sparse_mask,
           trace=False):
    am_zero = bool(np.all(np.asarray(attention_mask) == 0.0))
    key = ("nc", am_zero)
    if key not in _prog_cache:
        _prog_cache[key] = build_program(am_zero=am_zero)
    nc = _prog_cache[key]
    in_maps = _prepare_inputs(
        hidden_states, attention_mask, Wq, bq, Wk, bk, Wv, bv, sparse_mask
    )
    res = run_bass_kernel_spmd(nc, in_maps, list(range(8)), trace=trace)
    out = np.empty((2, L, HID), np.float32)
    for core in range(8):
        b, g = core // 4, core % 4
        arr = res.results[core]["out"]  # [4, 65, 4, 512]
        T = np.empty((65, 4, L - 1), np.float32)
        for qc in range(4):
            W = 512 if qc < 3 else 511
            T[:, :, 512 * qc : 512 * qc + W] = arr[qc][:, :, 0:W]
        for h in range(4):
            out[b][1:, 256 * g + 64 * h : 256 * g + 64 * h + 64] = (
                T[0:64, h, :] / T[64, h, :]
            ).T
    out[:, 0, :] = _host_row0(
        hidden_states, attention_mask, Wq, bq, Wk, bk, Wv, bv, sparse_mask
    )
    if trace:
        _prog_cache["last_results"] = res
    return out


# revision 18
# speedup vs baseline: 1.0349x; 1.0349x over previous
"""Sparse BertSelfAttention on 8 trn2 NeuronCores.

Sharding: core c -> batch b = c//4, head-group g = c%4 (heads 4g..4g+3).
Each core computes its batch's QT/KT/V projections for its 4 heads and the
sparse attention (local 128-band + global summary columns), producing the
output column block [2048, 256] for its (batch, head-group) in TRANSPOSED,
UNNORMALIZED form; the host divides by the softmax denominators and
transposes while assembling the full output.

Sparse structure (STRIDE=128, EXPR=8, L=2048, bidirectional):
  Query q's local key block is b*(q) = (q-1)//128 (q>=1); b*(0) = 0.
  So the SHIFTED query window w = [128w+1, 128w+129) attends key block w
  fully dense, plus the global summary columns:
    A: cols with (c mod 128) in 120..127  (128 cols)
    B: cols 128, 256, ..., 1920           (15 cols)
  Double-count handling: A-dup rows inside the local tile get a NEG bias
  folded into the Exp activation; the B-dup (local row 0, windows >= 1) is
  excluded by zeroing V row 0 of those blocks (adds 0 to numerator AND
  denominator). Query row 0 (keys = block 0 + globals) is computed on the
  HOST (tiny gemm over the ~271 allowed keys).

Layout: scores are computed transposed (S^T[k, q], keys on partitions) so
softmax denominators come from a ones-column appended to V. The context is
ALSO computed transposed -- ctx^T[d, q] = V-stationary matmuls with the
probability tiles as the moving operand -- which amortizes the global-A/B
contributions over 512 queries per matmul and makes the output DMA a
65-descriptor per-partition-contiguous transfer.
exp() skips max-subtraction: allowed scores are O(5), masked underflow to 0.

HW scheduling rules this file encodes (measured on the device, A/B in the
same benchmark run; the axon-tunnelled part drifts ~20% across runs so only
same-run comparisons were trusted):
 * Strided input DMAs are descriptor-generation-bound (~19ns/descriptor):
   wq/wv are host-packed to the exact SBUF layout so every input DMA is one
   descriptor per partition (input stream alone: ~45us -> ~6us/iter).
 * The transposed output turns 17 x 128-descriptor window stores into 4
   65-descriptor per-qc stores, and removes the reciprocal/normalize DVE
   work from the device entirely.
 * The gathered global K^T columns are plain columns of kt: gathered with
   4 DVE copies instead of 16 PE matmuls + 2 activations.
 * V projections run c-inner (8-chunk accumulation per block, 2-bank
   rotation); a 4-bank c-outer wave variant measured ~20% SLOWER end to
   end (v_mode="wave4" kept for reference). PV interleaves head pairs
   across two PSUM banks ("pair"), which beat per-head chains ("seq").
 * No per-iteration warmup matmuls: they occupied a PSUM bank used by the
   PV tail and serialized the iteration boundary.

Projections stream ht chunk-outer (c-outer) against 8 concurrent PSUM
accumulators so the PE starts as soon as the first 128-row chunk of ht
lands (each chunk's slice of Wk is packed in front of it so one DMA +
one semaphore covers both).
"""

import numpy as np
import ml_dtypes

import concourse.bass as bass
from concourse import bacc
import concourse.mybir as mybir
import concourse.tile as tile
from concourse.bass_utils import run_bass_kernel_spmd

BF16 = mybir.dt.bfloat16
F32 = mybir.dt.float32
AF = mybir.ActivationFunctionType

L = 2048
HID = 1024
NB = L // 128  # 16 key blocks / query windows
NEG = -10000.0

_prog_cache = {}


def _rep_attnB(v):
    out = np.zeros((128, 1), np.float32)
    for h in range(4):
        out[32 * h : 32 * h + 15, 0] = v
    return out


def _glob_cols():
    # A: (16 blocks) x (8 cols 120..127); B: 128,256,...,1920
    a = (np.arange(16)[:, None] * 128 + 120 + np.arange(8)[None, :]).reshape(-1)
    b = np.arange(1, 16) * 128
    return a, b


def _emit_body(nc, consts, pp, psum, parts, am_zero, ht_d, wq_d, wv_d, sm_d,
               out_d, warmup=False, v_mode="inner2", pv_mode="pair",
               dma2=False, ctx3=False, actsplit=False):
    # ---- input DMAs (all single-descriptor-per-partition) ----
    hwl = [
        consts.tile([128, 2304], BF16, tag=f"hw{c}", name=f"hw{c}")
        for c in range(8)
    ]
    htl = [hw[:, 256:2304] for hw in hwl]
    wq = consts.tile([128, 8, 256], BF16)
    wv = consts.tile([128, 8, 260], BF16)
    smalls = consts.tile([128, 6 + NB], F32)
    bqk = smalls[:, 0:4]
    aA = smalls[:, 4:5]
    aB = smalls[:, 5:6]
    bS = smalls[:, 6 : 6 + NB]

    for c in range(8):
        r = slice(128 * c, 128 * c + 128)
        if c == 0 and not dma2:
            nc.sync.dma_start(out=hwl[c][:, 0:1280], in_=ht_d[r, 0:1280])
            nc.sync.dma_start(out=hwl[c][:, 1280:2304], in_=ht_d[r, 1280:2304])
        else:
            nc.sync.dma_start(out=hwl[c], in_=ht_d[r, :])
    nc.sync.dma_start(out=wq.rearrange("p c n -> p (c n)"), in_=wq_d[:, :])
    nc.scalar.dma_start(out=wv.rearrange("p c n -> p (c n)"), in_=wv_d[:, :])
    nc.scalar.dma_start(out=smalls, in_=sm_d[:, :])

    # compact copies of ht's global summary columns (feed the vgA/vgB
    # projections; matmul operands need a single free dim).
    # cols 0:128 = A, 128:143 = B, 143:160 zero-pad.
    ghtAB = consts.tile([128, 8, 160], BF16)
    nc.vector.memset(ghtAB.rearrange("p a b -> p (a b)"), 0.0)
    for c in range(8):
        src = htl[c].rearrange("p (a b) -> p a b", b=128)
        nc.vector.tensor_copy(
            ghtAB[:, c, 0:128].rearrange("p (a b) -> p a b", b=8),
            src[:, :, 120:128],
        )
        nc.vector.tensor_copy(ghtAB[:, c, 128:143], src[:, 1:16, 0])

    def P(i):
        # 8 statically-tagged PSUM bank slots, reused across phases
        # (per-tag WAR deps give fine-grained overlap).
        return psum.tile([128, 512], F32, tag=f"P{i % 8}", name=f"P{i % 8}")

    # PE warmup (p-state ramp) while the first input chunks land.
    if warmup:
        dummy = consts.tile([128, 512], BF16)
        nc.vector.memset(dummy, 0.0)
        warm = P(7)
        for _ in range(5):
            nc.tensor.matmul(warm, lhsT=dummy[:, 0:128], rhs=dummy,
                             start=True, stop=True)

    # ---- K then Q projections, chunk-outer over ht ----
    qtl = [consts.tile([128, L], BF16, tag=f"qt{t}", name=f"qt{t}")
           for t in range(2)]
    ktl = [consts.tile([128, L], BF16, tag=f"kt{t}", name=f"kt{t}")
           for t in range(2)]
    for dstl, wsrc, bcol in (
        ((ktl, "hw", 2), (qtl, "wq", 0)) if "proj" in parts else ()
    ):
        tiles = {}
        for c in range(8):
            for t in range(2):
                for n in range(4):
                    if c == 0:
                        tiles[t, n] = P(4 * t + n)
                    lhsT = (
                        hwl[c][:, 128 * t : 128 * t + 128]
                        if wsrc == "hw"
                        else wq[:, c, 128 * t : 128 * t + 128]
                    )
                    nc.tensor.matmul(
                        tiles[t, n],
                        lhsT=lhsT,
                        rhs=htl[c][:, 512 * n : 512 * n + 512],
                        start=(c == 0),
                        stop=(c == 7),
                    )
        for t in range(2):
            for n in range(4):
                if actsplit and n % 2 == 1:
                    nc.vector.tensor_copy(
                        dstl[t][:, 512 * n : 512 * n + 512], tiles[t, n]
                    )
                else:
                    nc.scalar.activation(
                        dstl[t][:, 512 * n : 512 * n + 512],
                        tiles[t, n],
                        AF.Identity,
                        bias=bqk[:, bcol + t : bcol + t + 1],
                    )

    # ---- gathered global K^T (plain columns of kt) and global V ----
    ktgAB = consts.tile([128, 2, 160], BF16)
    vgA = consts.tile([128, 260], BF16)
    vgB = consts.tile([128, 260], BF16)
    if "ktg" in parts:
        for t in range(2):
            src = ktl[t].rearrange("p (a b) -> p a b", b=128)
            nc.vector.tensor_copy(
                ktgAB[:, t, 0:128].rearrange("p (a b) -> p a b", b=8),
                src[:, :, 120:128],
            )
            nc.vector.tensor_copy(ktgAB[:, t, 128:143], src[:, 1:16, 0])
            nc.vector.memset(ktgAB[:, t, 143:160], 0.0)

        # vgA/vgB pair-interleaved across two banks (same-bank
        # back-to-back accumulation stalls the PE)
        psa, psb = P(4), P(5)
        for c in range(8):
            nc.tensor.matmul(
                psa[:, 0:260], lhsT=ghtAB[:, c, 0:128], rhs=wv[:, c, :],
                start=(c == 0), stop=(c == 7),
            )
            nc.tensor.matmul(
                psb[0:32, 0:260], lhsT=ghtAB[:, c, 128:160], rhs=wv[:, c, :],
                start=(c == 0), stop=(c == 7),
            )
        nc.vector.tensor_copy(vgA, psa[:, 0:260])
        nc.vector.memset(
            vgA.rearrange("p (h d) -> p h d", d=65)[:, :, 64:65], 1.0
        )
        nc.vector.tensor_copy(vgB[0:15, :], psb[0:15, 0:260])
        nc.vector.memset(
            vgB[0:15, :].rearrange("p (h d) -> p h d", d=65)[:, :, 64:65], 1.0
        )
        q_r = nc.scalar if dma2 else nc.sync
        for h_ in range(1, 4):
            q_r.dma_start(
                out=vgB[32 * h_ : 32 * h_ + 15, :], in_=vgB[0:15, :]
            )

    # ---- attention: V waves, scores/exp, transposed PV ----
    vl = [consts.tile([128, 260], BF16, tag=f"v{blk}", name=f"v{blk}")
          for blk in range(NB)]
    outT = consts.tile([128, 4, 4, 512], F32, tag="outT", name="outT")
    ns = [0]
    scores_of = {}

    def emit_V(w, nbank=4):
        # nbank blocks per wave, c-outer across nbank banks
        blks = range(nbank * w, nbank * w + nbank)
        tiles = {}
        for c in range(8):
            for i, blk in enumerate(blks):
                if c == 0:
                    tiles[i] = P(i)
                nc.tensor.matmul(
                    tiles[i][:, 0:260],
                    lhsT=htl[c][:, 128 * blk : 128 * blk + 128],
                    rhs=wv[:, c, :],
                    start=(c == 0), stop=(c == 7),
                )
        for i, blk in enumerate(blks):
            nc.vector.tensor_copy(vl[blk], tiles[i][:, 0:260])
            nc.vector.memset(
                vl[blk].rearrange("p (h d) -> p h d", d=65)[:, :, 64:65], 1.0
            )
            # B-dup exclusion (key col 128*blk, windows >= 1) by zeroing
            # V row 0: adds 0 to numerator AND denominator, so the S-exp
            # bias is window-invariant
            if blk >= 1:
                nc.gpsimd.memset(vl[blk][0:1, :], 0.0)

    def emit_scores(qc):
        lo = 512 * qc + 1
        W = 512 if qc < 3 else 511
        slots = (4, 5, 0, 1)
        # B-scores for all 4 heads (packed on partition strips; unwritten
        # strips never read downstream)
        pgB = P(6)
        for h in range(4):
            t, hh = h // 2, h % 2
            p0 = 64 * hh
            nc.tensor.matmul(
                pgB[32 * h : 32 * h + 32, 0:W],
                lhsT=ktgAB[p0 : p0 + 64, t, 128:160],
                rhs=qtl[t][p0 : p0 + 64, lo : lo + W],
                start=True, stop=True,
                tile_position=(p0, 32 * h),
            )
        pB = pp.tile([128, 512], BF16, tag="pB")
        nc.scalar.activation(pB[:, 0:W], pgB[:, 0:W], AF.Exp, bias=aB)

        pAs, pSs = [], []
        for h in range(4):
            t, hh = h // 2, h % 2
            p0 = 64 * hh

            # global-A scores + exp
            pgA = P(slots[ns[0] % len(slots)])
            ns[0] += 1
            nc.tensor.matmul(
                pgA[:, 0:W],
                lhsT=ktgAB[p0 : p0 + 64, t, 0:128],
                rhs=qtl[t][p0 : p0 + 64, lo : lo + W],
                start=True, stop=True,
            )
            pA = pp.tile([128, 512], BF16, tag="pA")
            nc.scalar.activation(pA[:, 0:W], pgA[:, 0:W], AF.Exp, bias=aA)

            # local window scores: window w keys = block w
            pss = P(slots[ns[0] % len(slots)])
            ns[0] += 1
            for j in range(4):
                w_ = 4 * qc + j
                wW = 128 if w_ < 15 else 127
                nc.tensor.matmul(
                    pss[:, 128 * j : 128 * j + wW],
                    lhsT=ktl[t][p0 : p0 + 64, 128 * w_ : 128 * w_ + 128],
                    rhs=qtl[t][p0 : p0 + 64,
                               128 * w_ + 1 : 128 * w_ + 1 + wW],
                    start=True, stop=True,
                )
            pS = pp.tile([128, 512], BF16, tag="pS")
            if am_zero:
                # row-0 exclusion lives in the zeroed V rows, so one bias
                # column serves every window
                nc.scalar.activation(pS[:, 0:W], pss[:, 0:W], AF.Exp,
                                     bias=bS[:, 0:1])
            else:
                for j in range(4):
                    w_ = 4 * qc + j
                    wW = 128 if w_ < 15 else 127
                    nc.scalar.activation(
                        pS[:, 128 * j : 128 * j + wW],
                        pss[:, 128 * j : 128 * j + wW],
                        AF.Exp, bias=bS[:, w_ : w_ + 1],
                    )
            pAs.append(pA)
            pSs.append(pS)

        scores_of[qc] = (pB, pAs, pSs)

    def emit_pv(qc):
        # ctx^T[d, q]: V stationary, probs moving; head pairs interleave
        # across two banks so no same-bank back-to-back accumulation.
        pB, pAs, pSs = scores_of.pop(qc)
        W = 512 if qc < 3 else 511
        cslots = (7, 3, 2) if ctx3 else (7, 3)
        for hp in (0, 1):
            cxs = []
            for i, h in enumerate((2 * hp, 2 * hp + 1)):
                cxt = P(cslots[(2 * hp + i) % len(cslots)])
                cxs.append(cxt[0:65, 0:512])
            hs_order = (
                [(s_, i_) for s_ in range(6) for i_ in (0, 1)]
                if pv_mode == "pair" else
                [(s_, i_) for i_ in (0, 1) for s_ in range(6)]
            )
            for stage, i in hs_order:
                    h = 2 * hp + i
                    cx = cxs[i]
                    if stage == 0:
                        nc.tensor.matmul(
                            cx[:, 0:W],
                            lhsT=vgA[:, 65 * h : 65 * h + 65],
                            rhs=pAs[h][:, 0:W],
                            start=True, stop=False,
                            skip_group_check=True,
                        )
                    elif stage == 1:
                        nc.tensor.matmul(
                            cx[:, 0:W],
                            lhsT=vgB[32 * h : 32 * h + 15,
                                     65 * h : 65 * h + 65],
                            rhs=pB[32 * h : 32 * h + 15, 0:W],
                            start=False, stop=False,
                            tile_position=(32 * h, 0),
                            skip_group_check=True,
                        )
                    else:
                        j = stage - 2
                        w_ = 4 * qc + j
                        wW = 128 if w_ < 15 else 127
                        nc.tensor.matmul(
                            cx[:, 128 * j : 128 * j + wW],
                            lhsT=vl[w_][:, 65 * h : 65 * h + 65],
                            rhs=pSs[h][:, 128 * j : 128 * j + wW],
                            start=False, stop=(j == 3),
                            skip_group_check=True,
                        )
            for i, h in enumerate((2 * hp, 2 * hp + 1)):
                nc.vector.tensor_copy(outT[0:65, qc, h, 0:W], cxs[i][:, 0:W])
        q_ = nc.scalar if dma2 else (nc.sync if qc % 2 == 0 else nc.scalar)
        q_.dma_start(out=out_d[qc, :, :, :], in_=outT[0:65, qc, :, :])

    # V waves all run right after the projections: the last htl readers
    # then finish mid-iteration, so the next iteration's input DMAs overlap
    # the whole scores/PV tail (input stream is free in steady state).
    do_v = "v" in parts
    do_s = "scores" in parts
    do_p = do_s and "pv" in parts
    if do_v:
        if v_mode == "wave8":
            for w in range(2):
                emit_V(w, nbank=8)
        elif v_mode == "inner2":
            for blk in range(16):
                ps_ = P(blk % 2)
                for c in range(8):
                    nc.tensor.matmul(
                        ps_[:, 0:260],
                        lhsT=htl[c][:, 128 * blk : 128 * blk + 128],
                        rhs=wv[:, c, :],
                        start=(c == 0), stop=(c == 7),
                    )
                nc.vector.tensor_copy(vl[blk], ps_[:, 0:260])
                nc.vector.memset(
                    vl[blk].rearrange("p (h d) -> p h d", d=65)[:, :, 64:65],
                    1.0,
                )
                if blk >= 1:
                    nc.gpsimd.memset(vl[blk][0:1, :], 0.0)
        else:
            for w in range(4):
                emit_V(w)
    if do_s: emit_scores(0)
    if do_s: emit_scores(1)
    if do_p: emit_pv(0)
    if do_s: emit_scores(2)
    if do_p: emit_pv(1)
    if do_s: emit_scores(3)
    if do_p: emit_pv(2)
    if do_p: emit_pv(3)


def build_program(loop_n=None, am_zero=True, reps=1, warmup=None,
                  v_mode="inner2", pv_mode="pair", dma2=False, ctx3=False,
                  actsplit=False,
                  parts=("proj", "ktg", "v", "scores", "pv")):
    if warmup is None:
        # warmup matmuls (p-state ramp) pay off one-shot, but inside a loop
        # they re-run every iteration and WAR-couple the iteration boundary
        # through their PSUM bank: loop builds default them off.
        warmup = loop_n is None and reps == 1
    nc = bacc.Bacc(None)
    # ht chunks with the matching wk chunk packed in front: [wk 256 | ht 2048]
    ht_d = nc.dram_tensor("htwk", [HID, 2304], BF16, kind="ExternalInput")
    wq_d = nc.dram_tensor("wqp", [128, 8 * 256], BF16, kind="ExternalInput")
    wv_d = nc.dram_tensor("wvp", [128, 8 * 260], BF16, kind="ExternalInput")
    # smalls: cols 0:4 bqk | 4:5 attnA | 5:6 attnB | 6:22 biasS
    sm_d = nc.dram_tensor("smalls", [128, 6 + NB], F32, kind="ExternalInput")
    # out[qc][p][h][c] = ctx^T (d=p, head h, query 512*qc+1+c); p=64 rows are
    # the softmax denominators. Host divides + transposes.
    out_d = nc.dram_tensor("out", [4, 65, 4, 512], F32, kind="ExternalOutput")

    with tile.TileContext(nc) as tc:
        with (
            tc.tile_pool(name="consts", bufs=1) as consts,
            tc.tile_pool(name="pp", bufs=8) as pp,
            tc.tile_pool(name="ps", bufs=1, space="PSUM") as psum,
        ):
            import contextlib
            _lp = tc.For_i(0, loop_n, 1) if loop_n else contextlib.nullcontext()
            with _lp:
                for _rep in range(reps):
                    _emit_body(nc, consts, pp, psum, parts, am_zero,
                               ht_d, wq_d, wv_d, sm_d, out_d, warmup=warmup,
                               v_mode=v_mode, pv_mode=pv_mode, dma2=dma2,
                               ctx3=ctx3, actsplit=actsplit)
    nc.finalize()
    return nc


def _prepare_inputs(hidden_states, attention_mask, Wq, bq, Wk, bk, Wv, bv,
                    sparse_mask):
    bf = ml_dtypes.bfloat16
    hs = np.asarray(hidden_states, np.float32)
    am = np.asarray(attention_mask, np.float32).reshape(2, L)
    Wq = np.asarray(Wq, np.float32)
    Wk = np.asarray(Wk, np.float32)
    Wv = np.asarray(Wv, np.float32)
    bq = np.asarray(bq, np.float32)
    bk = np.asarray(bk, np.float32)
    gA, gB = _glob_cols()

    in_maps = []
    per_batch = {}
    for b in range(2):
        ht = hs[b].T.astype(np.float32)  # [1024, 2048]
        # per-window local bias: am over the window's key block, globals
        # excluded (A rows always; k=0 is in B for windows >= 1)
        bS = np.empty((128, NB), np.float32)
        for w in range(NB):
            col = am[b][128 * w : 128 * w + 128].copy()
            col[120:128] = NEG
            if w >= 1:
                col[0] = NEG
            bS[:, w] = col
        per_batch[b] = (
            ht,
            bS,
            am[b][gA].reshape(128, 1).copy(),
            _rep_attnB(am[b][gB]),
        )

    for core in range(8):
        b, g = core // 4, core % 4
        ht, bS, aAv, aBv = per_batch[b]
        cols = slice(256 * g, 256 * g + 256)
        # wq/wv packed chunk-major to the exact SBUF tile layout so the
        # input DMA is one descriptor per partition.
        wqp = np.ascontiguousarray(
            (Wq[:, cols] * 0.125).reshape(8, 128, 256).transpose(1, 0, 2)
            .reshape(128, 8 * 256)
        ).astype(bf)
        htwk = np.empty((HID, 2304), np.float32)
        htwk[:, 0:256] = Wk[:, cols]
        htwk[:, 256:2304] = ht
        wv_ = np.zeros((HID, 260), np.float32)
        for j in range(4):
            wv_[:, 65 * j : 65 * j + 64] = (
                Wv[:, cols.start + 64 * j : cols.start + 64 * j + 64]
            )
        wvp = np.ascontiguousarray(
            wv_.reshape(8, 128, 260).transpose(1, 0, 2).reshape(128, 8 * 260)
        ).astype(bf)
        bqk_ = np.stack(
            [
                bq[cols][:128] * 0.125,
                bq[cols][128:] * 0.125,
                bk[cols][:128],
                bk[cols][128:],
            ],
            axis=1,
        ).astype(np.float32)
        sm = np.empty((128, 6 + NB), np.float32)
        sm[:, 0:4] = bqk_
        sm[:, 4:5] = aAv
        sm[:, 5:6] = aBv
        sm[:, 6 : 6 + NB] = bS
        in_maps.append(
            dict(
                htwk=htwk.astype(bf),
                wqp=wqp,
                wvp=wvp,
                smalls=np.ascontiguousarray(sm),
            )
        )
    # NOTE: bv is folded nowhere: it is zeros by construction in this problem.
    assert np.all(np.asarray(bv) == 0.0), "kernel assumes zero V bias"
    return in_maps


def _host_row0(hidden_states, attention_mask, Wq, bq, Wk, bk, Wv, bv,
               sparse_mask):
    """Output row 0 (query position 0) for both batches, exact reference
    math restricted to the ~271 allowed keys of mask row 0."""
    hs = np.asarray(hidden_states, np.float32)
    am = np.asarray(attention_mask, np.float32).reshape(2, L)
    mask_row = np.asarray(sparse_mask, np.float32)[0]
    allowed = np.where(mask_row == 0.0)[0]
    Wq = np.asarray(Wq, np.float32)
    Wk = np.asarray(Wk, np.float32)
    Wv = np.asarray(Wv, np.float32)
    out0 = np.empty((2, HID), np.float32)
    for b in range(2):
        q0 = hs[b, 0] @ Wq + np.asarray(bq, np.float32)
        sub = hs[b][allowed]
        Ks = sub @ Wk + np.asarray(bk, np.float32)
        Vs = sub @ Wv + np.asarray(bv, np.float32)
        for h in range(16):
            sl = slice(64 * h, 64 * h + 64)
            s = np.clip(Ks[:, sl] @ q0[sl] / 8.0, -1e4, 1e4) + am[b][allowed]
            e = np.exp(s - s.max())
            out0[b, sl] = (e / e.sum()) @ Vs[:, sl]
    return out0


def kernel(hidden_states, attention_mask, Wq, bq, Wk, bk, Wv, bv, sparse_mask,
           trace=False):
    am_zero = bool(np.all(np.asarray(attention_mask) == 0.0))
    key = ("nc", am_zero)
    if key not in _prog_cache:
        _prog_cache[key] = build_program(am_zero=am_zero)
    nc = _prog_cache[key]
    in_maps = _prepare_inputs(
        hidden_states, attention_mask, Wq, bq, Wk, bk, Wv, bv, sparse_mask
    )
    res = run_bass_kernel_spmd(nc, in_maps, list(range(8)), trace=trace)
    out = np.empty((2, L, HID), np.float32)
    for core in range(8):
        b, g = core // 4, core % 4
        arr = res.results[core]["out"]  # [4, 65, 4, 512]
        T = np.empty((65, 4, L - 1), np.float32)
        for qc in range(4):
            W = 512 if qc < 3 else 511
            T[:, :, 512 * qc : 512 * qc + W] = arr[qc][:, :, 0:W]
        for h in range(4):
            out[b][1:, 256 * g + 64 * h : 256 * g + 64 * h + 64] = (
                T[0:64, h, :] / T[64, h, :]
            ).T
    out[:, 0, :] = _host_row0(
        hidden_states, attention_mask, Wq, bq, Wk, bk, Wv, bv, sparse_mask
    )
    if trace:
        _prog_cache["last_results"] = res
    return out
